# revision 2
# baseline (speedup 1.0000x reference)
"""Trainium2 Bass kernel v2 for nn_Backbone_3143916060887 (moe_routing).

Conv matmuls run as fp8e4m3 DoubleRow pairs with hi+lo residual splits on
both operands, dropping only the lo*lo term ("3term"): numerically ~bf16
(extra noise ~1e-3) at 0.75x the bf16 PE-row cost, and the DoubleRow pairing
itself doubles rows/cycle -> ~2.3x faster conv stack than the bf16 baseline.

Structure per layer (per core, 16 images):
  L1: 38 blocks of 3 output rows (y_pack=3, M = 3yo*32c), K-chunks 2x(18c x
      7y_rel = 126p), 15 DR matmuls per 4-image sg. x split host-side into
      (hi, 32*lo) fp8 planes; weight tiles (w*s, w*s/32) pair with them.
  L2: 28 blocks of 2 rows (M = 2yo*64c), chunks 2x(16c x 6y = 96p), 15 DR
      per 8-image sg. pool1 stores interleaved (hi, lo) fp8 planes (lo
      unscaled; pool values O(1) stay out of denormals).
  L3: 28 blocks (M = 128c), chunks (22,22,20)c x 5y_rel, 5 DR per kx.
Per-out-channel weight scales (absmax -> 240) keep fp8 in its normal range;
the activation's per-partition scale undoes them for free.

Pooling: the conv activation deinterleaves x into (phase, x/2) on its PSUM->
SBUF write so the x-maxpool is a packed-operand DVE TT (2x mode); y-maxpool
pairs partition groups via base-0 tiles. hi planes are written by Act-engine
copies and lo planes by Pool-engine subtracts to keep DVE under the PE pace.

Tail: router logits accumulate in PSUM during L3; softmax+top-2 coef is
computed locally pre-AG and shipped in the payload; feat goes out as
interleaved (hi, lo) fp8 rows in 3 AllGather chunks; experts run 3term-DR
over s-pairs; partials ReduceScatter back to token owners (L2 normalize).
"""

import numpy as np
import ml_dtypes

try:  # persistent XLA/NEFF cache so repeat processes skip the ~60s compile
    import jax
    jax.config.update("jax_compilation_cache_dir", "/tmp/jax_cache")
    jax.config.update("jax_persistent_cache_min_entry_size_bytes", -1)
    jax.config.update("jax_persistent_cache_min_compile_time_secs", 0.0)
except Exception:
    pass

import concourse.bass as bass
import concourse.bacc as bacc
import concourse.mybir as mybir
import concourse.tile as tile
from concourse.bass_utils import run_bass_kernel_spmd

NCORE = 8
B, CIN, H, W = 128, 36, 112, 112
BL = B // NCORE
E, F, HID, D = 8, 7 * 7 * 128, 512, 256
BN_EPS = 1e-5
F8 = ml_dtypes.float8_e4m3fn
BF16 = ml_dtypes.bfloat16
RELU = mybir.ActivationFunctionType.Relu
COPY = mybir.ActivationFunctionType.Copy
MAX = mybir.AluOpType.max
ADD = mybir.AluOpType.add
SUB = mybir.AluOpType.subtract
DR = mybir.MatmulPerfMode.DoubleRow

NB1 = 38            # L1 blocks (3 y-rows each; 38*3 = 114 >= 112)
L3_CH = [(0, 22), (22, 22), (44, 20)]
NROW = 102          # AG payload rows: 2s/2s+1 hi/lo for s<49, 98/99 coef


# ---------------------------------------------------------------------------
# host-side weight preparation
# ---------------------------------------------------------------------------

def _fold_bn(w, b, g, beta, m, v):
    s = g.astype(np.float64) / np.sqrt(v.astype(np.float64) + BN_EPS)
    return (w.astype(np.float64) * s[:, None, None, None],
            (b.astype(np.float64) - m.astype(np.float64)) * s + beta.astype(np.float64))


def _q8(x):
    return x.astype(F8)


def _scales(w):
    amax = np.abs(w.reshape(w.shape[0], -1)).max(axis=1)
    return 240.0 / np.maximum(amax, 1e-30)


def _l1_weights(w):
    """w float64 [32, 36, 5, 5] -> (w1hh [126,5,2,2,96], w1lh [126,5,2,96],
    s1inv [96,1]).  p=(c_r*7+y_rel), m=(yo*32+c)."""
    s = _scales(w)
    ws = w * s[:, None, None, None]
    full = np.zeros((126, 5, 2, 96), np.float64)
    for kx in range(5):
        for j in range(2):
            for c_r in range(18):
                for y_rel in range(7):
                    p = c_r * 7 + y_rel
                    for yo in range(3):
                        ky = y_rel - yo
                        if 0 <= ky < 5:
                            full[p, kx, j, yo * 32:(yo + 1) * 32] = \
                                ws[:, j * 18 + c_r, ky, kx]
    wh = _q8(full)
    wl = _q8(full - wh.astype(np.float64))
    w1hh = np.zeros((126, 5, 2, 2, 96), F8)
    w1hh[:, :, :, 0, :] = wh
    w1hh[:, :, :, 1, :] = _q8(full / 32.0)
    s1inv = np.tile(1.0 / s, 3).astype(np.float32)[:, None]
    return w1hh, np.ascontiguousarray(wl), s1inv


def _l2_weights(w):
    """w float64 [64, 32, 5, 5] -> (w2hh [2ph,128,5,2,2,128],
    w2lh [2ph,128,5,2,128], s2inv [128,1]).  p=(c_r*8+y_rel), m=(yo*64+c);
    phase ph shifts the valid window by 2 rows (joint 2-block gather)."""
    s = _scales(w)
    ws = w * s[:, None, None, None]
    full = np.zeros((2, 128, 5, 2, 128), np.float64)
    for ph in range(2):
        for kx in range(5):
            for j in range(2):
                for c_r in range(16):
                    for yo in range(2):
                        for ky in range(5):
                            y_rel = 2 * ph + yo + ky
                            p = c_r * 8 + y_rel
                            full[ph, p, kx, j, yo * 64:(yo + 1) * 64] = \
                                ws[:, j * 16 + c_r, ky, kx]
    wh = _q8(full)
    wl = _q8(full - wh.astype(np.float64))
    w2hh = np.zeros((2, 128, 5, 2, 2, 128), F8)
    w2hh[:, :, :, :, 0, :] = wh
    w2hh[:, :, :, :, 1, :] = wh          # device lo is unscaled
    s2inv = np.tile(1.0 / s, 2).astype(np.float32)[:, None]
    return w2hh, np.ascontiguousarray(wl), s2inv


def _l3_weights(w):
    """-> (w3hh [2ph,96,5,2gp,2g,128], w3lh same, s3inv).
    p=(c_r*6+y_rel), chunks g of 16 cin; phase shifts window by 1 row."""
    s = _scales(w)
    ws = w * s[:, None, None, None]
    full = np.zeros((2, 4, 96, 5, 128), np.float64)
    for ph in range(2):
        for g in range(4):
            for kx in range(5):
                for c_r in range(16):
                    for ky in range(5):
                        y_rel = ph + ky
                        full[ph, g, c_r * 6 + y_rel, kx, :] = \
                            ws[:, 16 * g + c_r, ky, kx]
    wh = _q8(full)
    wl = _q8(full - wh.astype(np.float64))
    # pair chunks (g0,g1), (g2,g3) along the DR k-tile dim
    w3hh = np.ascontiguousarray(
        wh.transpose(0, 2, 3, 1, 4).reshape(2, 96, 5, 2, 2, 128))
    w3lh = np.ascontiguousarray(
        wl.transpose(0, 2, 3, 1, 4).reshape(2, 96, 5, 2, 2, 128))
    s3inv = (1.0 / s).astype(np.float32)[:, None]
    return w3hh, w3lh, s3inv


def _l1_blocks(xc):
    """xc [16, 36, 112, 112] f32 -> [36, 118, 2, 16, 116] fp8 (c, y, t, i, x):
    t=0 hi, t=1 32*lo."""
    hi = xc.astype(F8)
    lo32 = ((xc - hi.astype(np.float32)) * 32.0).astype(F8)
    out = np.zeros((36, 118, 2, BL, 116), F8)
    out[:, 2:114, 0, :, 2:114] = hi.transpose(1, 2, 0, 3)
    out[:, 2:114, 1, :, 2:114] = lo32.transpose(1, 2, 0, 3)
    return np.ascontiguousarray(out)


def _expert_weights(ew1, eb1, ew2, eb2):
    """ew1 [F, HID] -> hi/lo [128, 50, 4m, 128] fp8 (0.25 avgpool + per-hid
    scale folded); (ew1h, ew1l, es_inv [128,4], ew2t bf16, eb1t, eb2r)."""
    w = 0.25 * ew1.astype(np.float64)
    amax = np.abs(w).max(axis=0)
    s = 240.0 / np.maximum(amax, 1e-30)
    ws = w * s[None, :]
    wh = ws.astype(F8)
    wl = (ws - wh.astype(np.float64)).astype(F8)

    def pack(t):
        # feat feature index f = c*49 + s (c-major), matching the router pack
        return np.ascontiguousarray(t.reshape(128, 49, 4, 128))

    es_inv = np.ascontiguousarray((1.0 / s).reshape(4, 128).T.astype(np.float32))
    ew2t = np.ascontiguousarray(
        ew2.reshape(4, 128, 256).transpose(1, 0, 2).astype(BF16))
    eb1t = np.ascontiguousarray(eb1.reshape(4, 128).T.astype(np.float32))
    eb2r = eb2.astype(BF16)[None, :]
    return pack(wh), pack(wl), es_inv, ew2t, eb1t, eb2r


# ---------------------------------------------------------------------------
# fused module v2
# ---------------------------------------------------------------------------

def build_fused_v2(warm_count=0, debug=False):
    nc = bacc.Bacc("TRN2", target_bir_lowering=False, debug=False,
                   num_devices=NCORE)
    f32, bf, f8 = mybir.dt.float32, mybir.dt.bfloat16, mybir.dt.float8e4
    xblk_d = nc.dram_tensor("xblk", [36, 118, 2, BL, 116], f8, kind="ExternalInput")
    w1hh_d = nc.dram_tensor("w1hh", [126, 5, 2, 2, 96], f8, kind="ExternalInput")
    w1lh_d = nc.dram_tensor("w1lh", [126, 5, 2, 96], f8, kind="ExternalInput")
    w2hh_d = nc.dram_tensor("w2hh", [2, 128, 5, 2, 2, 128], f8, kind="ExternalInput")
    w2lh_d = nc.dram_tensor("w2lh", [2, 128, 5, 2, 128], f8, kind="ExternalInput")
    w3hh_d = nc.dram_tensor("w3hh", [2, 96, 5, 2, 2, 128], f8, kind="ExternalInput")
    w3lh_d = nc.dram_tensor("w3lh", [2, 96, 5, 2, 2, 128], f8, kind="ExternalInput")
    b1_d = nc.dram_tensor("b1", [96, 1], f32, kind="ExternalInput")
    b2_d = nc.dram_tensor("b2", [128, 1], f32, kind="ExternalInput")
    b3_d = nc.dram_tensor("b3", [128, 1], f32, kind="ExternalInput")
    s1_d = nc.dram_tensor("s1inv", [96, 1], f32, kind="ExternalInput")
    s2_d = nc.dram_tensor("s2inv", [128, 1], f32, kind="ExternalInput")
    s3_d = nc.dram_tensor("s3inv", [128, 1], f32, kind="ExternalInput")
    rwt_d = nc.dram_tensor("rwt", [128, 49, 8], bf, kind="ExternalInput")
    rb_d = nc.dram_tensor("rb", [1, 8], f32, kind="ExternalInput")
    ew1h_d = nc.dram_tensor("ew1h", [128, 49, 4, 128], f8, kind="ExternalInput")
    ew1l_d = nc.dram_tensor("ew1l", [128, 49, 4, 128], f8, kind="ExternalInput")
    esi_d = nc.dram_tensor("esi", [128, 4], f32, kind="ExternalInput")
    ew2_d = nc.dram_tensor("ew2", [128, 4, 256], bf, kind="ExternalInput")
    eb1_d = nc.dram_tensor("eb1", [128, 4], f32, kind="ExternalInput")
    eb2r_d = nc.dram_tensor("eb2r", [1, 256], bf, kind="ExternalInput")
    oh_d = nc.dram_tensor("oh", [128, 8], f32, kind="ExternalInput")
    y_d = nc.dram_tensor("y", [BL, 256], f32, kind="ExternalOutput")
    if debug:
        dbg_pool1 = nc.dram_tensor("dbg_pool1", [128, 60, 2, 4, 60], f8, kind="ExternalOutput")
        dbg_pool2 = nc.dram_tensor("dbg_pool2", [128, 32, 2, 8, 32], f8, kind="ExternalOutput")
        dbg_feat = nc.dram_tensor("dbg_feat", [128, 49, BL], mybir.dt.bfloat16, kind="ExternalOutput")
        dbg_fz = nc.dram_tensor("dbg_fz", [128, NROW, BL], f8, kind="ExternalOutput")
        dbg_lg = nc.dram_tensor("dbg_lg", [BL, 8], f32, kind="ExternalOutput")
        dbg_coefe = nc.dram_tensor("dbg_coefe", [128, 1], f32, kind="ExternalOutput")
        dbg_hid = nc.dram_tensor("dbg_hid", [128, 4, 128], mybir.dt.bfloat16, kind="ExternalOutput")
        dbg_featT = nc.dram_tensor("dbg_featT", [128, NCORE, 49, 2, BL], f8, kind="ExternalOutput")

    ag_d = [nc.dram_tensor(f"ag{i}", [128, n, BL], f8, kind="Internal")
            for i, n in enumerate((28, 28, 28, 18))]
    cc_d = [nc.dram_tensor(f"cc{i}", [NCORE, 128, n, BL], f8,
                           kind="Internal", addr_space="Shared")
            for i, n in enumerate((28, 28, 28, 18))]
    rs_in = nc.dram_tensor("rs_in", [128, 256], f32, kind="Internal")
    rs_out = nc.dram_tensor("rs_out", [BL, 256], f32, kind="Internal")
    RG = [list(range(NCORE))]

    from contextlib import ExitStack
    with tile.TileContext(nc) as tc, ExitStack() as ctx:
        wp = ctx.enter_context(tc.tile_pool(name="weights", bufs=1))
        xp_pool = ctx.enter_context(tc.tile_pool(name="xblk", bufs=2))
        blkp = ctx.enter_context(tc.tile_pool(name="blk", bufs=3))
        psp = ctx.enter_context(tc.tile_pool(name="psum", bufs=2, space="PSUM"))
        psq = ctx.enter_context(tc.tile_pool(name="psum2", bufs=1, space="PSUM"))
        vp = ctx.enter_context(tc.tile_pool(name="vtmp", bufs=4))
        pers = ctx.enter_context(tc.tile_pool(name="persist", bufs=1))

        w1hh = wp.tile([126, 5, 2, 2, 96], f8)
        w1lh = wp.tile([126, 5, 2, 96], f8)
        w2hh = wp.tile([128, 2, 5, 2, 2, 128], f8)
        w2lh = wp.tile([128, 2, 5, 2, 128], f8)
        w3hh = wp.tile([96, 2, 5, 2, 2, 128], f8)
        w3lh = wp.tile([96, 2, 5, 2, 2, 128], f8)
        b1t = wp.tile([96, 1], f32)
        b2t = wp.tile([128, 1], f32)
        b3t = wp.tile([128, 1], f32)
        s1t = wp.tile([96, 1], f32)
        s2t = wp.tile([128, 1], f32)
        s3t = wp.tile([128, 1], f32)
        for dst, src in ((w1hh, w1hh_d), (w1lh, w1lh_d), (b1t, b1_d),
                         (s1t, s1_d)):
            nc.scalar.dma_start(dst[:], src[:])

        # act-table preheat: Exp's set also serves Relu/Copy, so the softmax
        # later needs no mid-kernel table load
        preheat = wp.tile([1, 1], f32)
        nc.scalar.activation(out=preheat[:], in_=s1t[0:1, :],
                             func=mybir.ActivationFunctionType.Exp)

        # persistent pools, (group, channel)-partitioned, hi/lo interleaved
        pool1 = pers.tile([128, 60, 2, 4, 60], f8)    # (q4*c32), y, t, i4, x
        pool2 = pers.tile([128, 32, 2, 8, 32], f8)    # (h2*c64), y, t, i8, x
        feat_sb = pers.tile([128, 49, BL], bf)
        fz = pers.tile([128, NROW, BL], f8)
        for t, n in ((pool1, 60), (pool2, 32)):
            nc.vector.memset(t[:, 0:2], 0.0)
            nc.vector.memset(t[:, n - 2:n], 0.0)
            nc.vector.memset(t[:, :, :, :, 0:2], 0.0)
            nc.vector.memset(t[:, :, :, :, n - 2:n], 0.0)

        # PSUM tail tiles created first so each matmul target is bank-aligned
        accum = psq.tile([128, 4, 128], mybir.dt.float32, tag="accum")
        ps2 = psq.tile([128, 256], mybir.dt.float32, tag="ps2")
        ps_r = psq.tile([BL, 8], mybir.dt.float32, tag="psr")

        # expert weights streamed during L1 (pinned by WAW dep)
        ew1h = wp.tile([128, 49, 4, 128], f8)
        ew1l = wp.tile([128, 49, 4, 128], f8)
        EW1_Q = [0, 13, 25, 37, 49]
        EW1_SCHED = {14 + 2 * k: k for k in range(8)}

        featT = pers.tile([128, NCORE, 49, 2, BL], f8)   # c, r, s, hl, t

        # ---------------- layer 1: 36 -> 32, 112x112 -> pool 56x56
        L1Q0 = (nc.sync, nc.sync, nc.gpsimd, nc.gpsimd)
        L1Q = (nc.sync, nc.sync, nc.scalar, nc.scalar)
        carry = None
        prow_written = 0
        for b in range(NB1):
            xt = xp_pool.tile([126, 2, 2, BL, 116], f8)
            for j in range(2):
                for t in range(2):
                    (L1Q0 if b < 2 else L1Q)[2 * j + t].dma_start(
                        xt[:, j, t],
                        xblk_d[18 * j:18 * j + 18, 3 * b:3 * b + 7, t])
            if b == 1:
                for dst2, src2 in ((b2t, b2_d), (s2t, s2_d), (b3t, b3_d),
                                   (s3t, s3_d)):
                    nc.gpsimd.dma_start(dst2[:], src2[:])
                nc.gpsimd.dma_start(
                    w2hh[:], w2hh_d[:].rearrange("ph p a b c d -> p ph a b c d"))
                nc.gpsimd.dma_start(
                    w2lh[:], w2lh_d[:].rearrange("ph p a b c -> p ph a b c"))
            if b == 3:
                nc.gpsimd.dma_start(
                    w3hh[:], w3hh_d[:].rearrange("ph p a b c d -> p ph a b c d"))
                nc.gpsimd.dma_start(
                    w3lh[:], w3lh_d[:].rearrange("ph p a b c d -> p ph a b c d"))
            ev = vp.tile([96, 4, 4, 2, 56], bf, tag="ev1", bufs=2)
            for half in range(2):
                ps = psp.tile([128, 2, 512], mybir.dt.float32, name="ps", tag="ps")
                for sgk in range(2):
                    sg = 2 * half + sgk
                    dst = ps[0:96, sgk, 0:448].rearrange("p (i x) -> p i x", i=4)
                    k = 0
                    for kx in range(5):
                        for j in range(2):
                            nc.tensor.matmul(
                                dst, w1hh[:, kx, j],
                                xt[:, j, :, 4 * sg:4 * sg + 4, kx:kx + 112],
                                start=(k == 0), stop=False, perf_mode=DR)
                            k += 1
                        nc.tensor.matmul(
                            dst, w1lh[:, kx],
                            xt[:, :, 0, 4 * sg:4 * sg + 4, kx:kx + 112],
                            start=False, stop=(kx == 4), perf_mode=DR)
                nc.scalar.activation(
                    out=ev[:, 2 * half:2 * half + 2]
                    .rearrange("c s i ph xp -> c (s i) xp ph"),
                    in_=ps[0:96, :, 0:448], func=RELU, bias=b1t[:], scale=s1t[:])
            evg = ev[:].rearrange("c s i ph xp -> c (s i) ph xp")
            xpo = vp.tile([32, 3, 16, 56], bf, tag="xpo1", bufs=2)
            nrow = min(3, 112 - 3 * b)
            for yo in range(nrow):
                nc.vector.tensor_tensor(
                    xpo[:, yo], evg[32 * yo:32 * yo + 32, :, 0, :],
                    evg[32 * yo:32 * yo + 32, :, 1, :], MAX)

            ylist = ([carry] if carry is not None else []) + \
                [(xpo, yo) for yo in range(nrow)]
            pairs = []
            while len(ylist) >= 2:
                pairs.append((ylist[0], ylist[1]))
                ylist = ylist[2:]
            carry = ylist[0] if ylist else None
            for (ta, ya), (tb, yb) in pairs:
                prow = prow_written
                prow_written += 1
                vrow = vp.tile([32, 16, 56], bf, tag="vrow", bufs=2)
                nc.vector.tensor_tensor(vrow[:], ta[:, ya], tb[:, yb], MAX)
                hrow = vp.tile([32, 16, 56], f8, tag="hrow", bufs=2)
                nc.scalar.activation(out=hrow[:], in_=vrow[:], func=COPY)
                for q in range(4):
                    nc.vector.tensor_copy(
                        pool1[32 * q:32 * q + 32, 2 + prow, 0, :, 2:58],
                        hrow[:, 4 * q:4 * q + 4, :])
                    nc.gpsimd.tensor_tensor(
                        pool1[32 * q:32 * q + 32, 2 + prow, 1, :, 2:58],
                        vrow[:, 4 * q:4 * q + 4, :], hrow[:, 4 * q:4 * q + 4, :],
                        SUB)
            if b in EW1_SCHED:
                k = EW1_SCHED[b]
                src_t, dst_t = (ew1h_d, ew1h) if k < 4 else (ew1l_d, ew1l)
                s0, s1 = EW1_Q[k % 4], EW1_Q[k % 4 + 1]
                nc.vector.tensor_copy(
                    dst_t[0:1, s0:s0 + 1, 0:1, 0:1]
                    .rearrange("c s m h -> c (s m h)"),
                    pool1[0:1, 1 + prow_written:2 + prow_written, 0:1, 0:1, 2:3]
                    .rearrange("c y t i x -> c (y t i x)"))
                nc.sync.dma_start(dst_t[:, s0:s1], src_t[:, s0:s1])

        # ---------------- layer 2: 32 -> 64, 56x56 -> pool 28x28
        L2Q = (nc.sync, nc.sync, nc.scalar, nc.scalar,
               nc.sync, nc.sync, nc.scalar, nc.scalar)
        for bb in range(14):
            bt = blkp.tile([128, 2, 2, 16, 60], f8, tag="bt2", bufs=2)
            for j in range(2):
                for q in range(4):
                    src = pool1[32 * q + 16 * j:32 * q + 16 * j + 16,
                                4 * bb:4 * bb + 8]
                    L2Q[4 * j + q].dma_start(
                        bt[:, j, :, 4 * q:4 * q + 4, :], src)
            for sub in range(2):
                b = 2 * bb + sub
                ev2 = vp.tile([128, 2, 8, 2, 28], bf, tag="ev2", bufs=1)
                ps = psp.tile([128, 2, 512], mybir.dt.float32, name="ps", tag="ps")
                for sg in range(2):
                    dst = ps[:, sg, 0:448].rearrange("p (i x) -> p i x", i=8)
                    k = 0
                    for kx in range(5):
                        for j in range(2):
                            nc.tensor.matmul(
                                dst, w2hh[:, sub, kx, j],
                                bt[:, j, :, 8 * sg:8 * sg + 8, kx:kx + 56],
                                start=(k == 0), stop=False, perf_mode=DR)
                            k += 1
                        nc.tensor.matmul(
                            dst, w2lh[:, sub, kx],
                            bt[:, :, 0, 8 * sg:8 * sg + 8, kx:kx + 56],
                            start=False, stop=(kx == 4), perf_mode=DR)
                nc.scalar.activation(
                    out=ev2[:].rearrange("c s i ph xp -> c (s i) xp ph"),
                    in_=ps[:, :, 0:448], func=RELU, bias=b2t[:], scale=s2t[:])
                evg2 = ev2[:].rearrange("c s i ph xp -> c (s i) ph xp")
                xpo2 = vp.tile([64, 2, 16, 28], bf, tag="xpo2", bufs=2)
                for yo in range(2):
                    nc.vector.tensor_tensor(
                        xpo2[:, yo], evg2[64 * yo:64 * yo + 64, :, 0, :],
                        evg2[64 * yo:64 * yo + 64, :, 1, :], MAX)
                vrow2 = vp.tile([64, 16, 28], bf, tag="vrow2", bufs=2)
                nc.vector.tensor_tensor(vrow2[:], xpo2[:, 0], xpo2[:, 1], MAX)
                hrow2 = vp.tile([64, 16, 28], f8, tag="hrow2", bufs=2)
                nc.scalar.activation(out=hrow2[:], in_=vrow2[:], func=COPY)
                for h in range(2):
                    nc.vector.tensor_copy(
                        pool2[64 * h:64 * h + 64, 2 + b, 0, :, 2:30],
                        hrow2[:, 8 * h:8 * h + 8, :])
                    nc.gpsimd.tensor_tensor(
                        pool2[64 * h:64 * h + 64, 2 + b, 1, :, 2:30],
                        vrow2[:, 8 * h:8 * h + 8, :],
                        hrow2[:, 8 * h:8 * h + 8, :], SUB)

        # router weights + expert tail constants
        rwt = wp.tile([128, 49, 8], bf)
        rbt = wp.tile([1, 8], f32)
        ones16 = wp.tile([1, BL], f32)
        ones = wp.tile([1, 128], bf)
        zf8 = wp.tile([1, 512], f8)
        nc.vector.memset(zf8[:], 0.0)
        oht = wp.tile([128, 8], f32)
        nc.scalar.dma_start(rwt[:], rwt_d[:])
        nc.scalar.dma_start(rbt[:], rb_d[:])
        nc.scalar.dma_start(oht[:], oh_d[:])
        nc.vector.memset(ones[:], 1.0)
        nc.vector.memset(ones16[:], 1.0)

        ew2t = wp.tile([128, 4, 256], bf)
        eb1t = wp.tile([128, 4], f32)
        esit = wp.tile([128, 4], f32)
        eb2t = wp.tile([1, 256], bf)
        nc.scalar.dma_start(ew2t[:], ew2_d[:])
        nc.scalar.dma_start(eb1t[:], eb1_d[:])
        nc.scalar.dma_start(esit[:], esi_d[:])
        nc.scalar.dma_start(eb2t[:], eb2r_d[:])

        # ---------------- layer 3: 64 -> 128, 28x28 conv + streamed pools
        L3Q = (nc.sync, nc.scalar, nc.sync, nc.scalar,
               nc.sync, nc.scalar, nc.sync, nc.scalar)
        xr_prev = avx_prev = None
        for b in range(28):
            bb, sub = divmod(b, 2)
            if sub == 0:
                bt3 = blkp.tile([96, 4, 2, 16, 32], f8, tag="bt3", bufs=2)
                for g in range(4):
                    for h in range(2):
                        src = pool2[64 * h + 16 * g:64 * h + 16 * g + 16,
                                    2 * bb:2 * bb + 6]
                        L3Q[2 * g + h].dma_start(
                            bt3[:, g, :, 8 * h:8 * h + 8, :], src)
            ps3 = psp.tile([128, 2, 512], mybir.dt.float32, name="ps", tag="ps")
            dst = ps3[:, 0, 0:448].rearrange("p (i x) -> p i x", i=16)
            for kx in range(5):
                for gp in range(2):
                    nc.tensor.matmul(dst, w3hh[:, sub, kx, gp],
                                     bt3[:, 2 * gp:2 * gp + 2, 0, :, kx:kx + 28],
                                     start=(kx == 0 and gp == 0), stop=False,
                                     perf_mode=DR)
                    nc.tensor.matmul(dst, w3hh[:, sub, kx, gp],
                                     bt3[:, 2 * gp:2 * gp + 2, 1, :, kx:kx + 28],
                                     start=False, stop=False, perf_mode=DR)
                    nc.tensor.matmul(dst, w3lh[:, sub, kx, gp],
                                     bt3[:, 2 * gp:2 * gp + 2, 0, :, kx:kx + 28],
                                     start=False,
                                     stop=(kx == 4 and gp == 1), perf_mode=DR)
            ev3 = vp.tile([128, 16, 2, 14], mybir.dt.float32, tag="ev3", bufs=1)
            nc.scalar.activation(
                out=ev3[:].rearrange("c i ph xp -> c i xp ph"),
                in_=ps3[:, 0, 0:448], func=RELU, bias=b3t[:], scale=s3t[:])
            xr = vp.tile([128, BL, 14], mybir.dt.float32,
                         tag=f"xr{b % 2}", bufs=2, name=f"xr_{b}")
            nc.vector.tensor_tensor(xr[:], ev3[:, :, 0, :], ev3[:, :, 1, :], MAX)
            if b % 2 == 0:
                xr_prev = xr
            else:
                yp = (b - 1) // 2
                mrow = vp.tile([128, BL, 14], mybir.dt.float32, tag="mrow",
                               bufs=1, name=f"mrow_{b}")
                nc.vector.tensor_tensor(mrow[:], xr_prev[:], xr[:], MAX)
                avx = vp.tile([128, BL, 7], mybir.dt.float32,
                              tag=f"av{yp % 2}", bufs=2, name=f"avx_{b}")
                nc.vector.tensor_tensor(avx[:], mrow[:, :, 0::2],
                                        mrow[:, :, 1::2], ADD)
                if yp % 2 == 0:
                    avx_prev = avx
                else:
                    yr = (yp - 1) // 2
                    nc.vector.tensor_tensor(
                        feat_sb[:, 7 * yr:7 * yr + 7, :]
                        .rearrange("c s t -> c t s"),
                        avx_prev[:], avx[:], ADD)
                    nc.scalar.activation(
                        out=fz[:, 14 * yr:14 * yr + 14:2, :],
                        in_=feat_sb[:, 7 * yr:7 * yr + 7, :], func=COPY)
                    nc.vector.tensor_tensor(
                        fz[:, 14 * yr + 1:14 * yr + 15:2, :],
                        feat_sb[:, 7 * yr:7 * yr + 7, :],
                        fz[:, 14 * yr:14 * yr + 14:2, :], SUB)
            if 17 <= b <= 24:
                r = b - 17
                nc.gpsimd.dma_start(
                    featT[:, r, 0:14],
                    cc_d[0][r].rearrange("c (s l) t -> c s l t", l=2))
            if 25 <= b <= 27:
                r = b - 25
                nc.gpsimd.dma_start(
                    featT[:, r, 14:28],
                    cc_d[1][r].rearrange("c (s l) t -> c s l t", l=2))
            if b >= 5 and (b - 5) % 4 == 0:
                yrr = (b - 5) // 4
                for s in range(7 * yrr, 7 * yrr + 7):
                    nc.tensor.matmul(ps_r[:], feat_sb[:, s, :], rwt[:, s, :],
                                     start=(s == 0), stop=False)
            if b == 8:
                nc.scalar.dma_start(ag_d[0][:], fz[:, 0:28, :])
                nc.gpsimd.collective_compute(
                    "AllGather", mybir.AluOpType.bypass, replica_groups=RG,
                    ins=[ag_d[0][:]], outs=[cc_d[0][:]])
            if b == 16:
                nc.scalar.dma_start(ag_d[1][:], fz[:, 28:56, :])
                nc.gpsimd.collective_compute(
                    "AllGather", mybir.AluOpType.bypass, replica_groups=RG,
                    ins=[ag_d[1][:]], outs=[cc_d[1][:]])
            if b == 24:
                nc.scalar.dma_start(ag_d[2][:], fz[:, 56:84, :])
                nc.gpsimd.collective_compute(
                    "AllGather", mybir.AluOpType.bypass, replica_groups=RG,
                    ins=[ag_d[2][:]], outs=[cc_d[2][:]])

        # ---------------- router tail + local softmax top-2 -> coef row
        for s in range(42, 49):
            nc.tensor.matmul(ps_r[:], feat_sb[:, s, :], rwt[:, s, :],
                             start=False, stop=False)
        nc.tensor.matmul(ps_r[:], ones16[:], rbt[:], start=False, stop=True)
        lg = vp.tile([BL, 8], f32, tag="lg", bufs=1)
        nc.vector.tensor_copy(lg[:], ps_r[:])
        m1 = vp.tile([BL, 1], f32, tag="m1", bufs=1)
        nc.vector.tensor_reduce(m1[:], lg[:], axis=mybir.AxisListType.X, op=MAX)
        negm1 = vp.tile([BL, 1], f32, tag="negm1", bufs=1)
        nc.vector.tensor_scalar_mul(negm1[:], m1[:], -1.0)
        eqm = vp.tile([BL, 8], f32, tag="eqm", bufs=1)
        nc.vector.tensor_scalar(eqm[:], lg[:], m1[:], None,
                                op0=mybir.AluOpType.is_equal)
        pen = vp.tile([BL, 8], f32, tag="pen", bufs=1)
        nc.vector.tensor_scalar_mul(pen[:], eqm[:], -1e30)
        msk = vp.tile([BL, 8], f32, tag="msk", bufs=1)
        nc.vector.tensor_tensor(msk[:], lg[:], pen[:], ADD)
        m2 = vp.tile([BL, 1], f32, tag="m2", bufs=1)
        nc.vector.tensor_reduce(m2[:], msk[:], axis=mybir.AxisListType.X, op=MAX)
        z = vp.tile([BL, 8], f32, tag="z", bufs=1)
        nc.scalar.activation(out=z[:], in_=lg[:],
                             func=mybir.ActivationFunctionType.Exp,
                             bias=negm1[:], scale=1.0)
        e2 = vp.tile([BL, 1], f32, tag="e2", bufs=1)
        nc.scalar.activation(out=e2[:], in_=m2[:],
                             func=mybir.ActivationFunctionType.Exp,
                             bias=negm1[:], scale=1.0)
        den = vp.tile([BL, 1], f32, tag="den", bufs=1)
        nc.vector.tensor_scalar_add(den[:], e2[:], 1.0)
        rden = vp.tile([BL, 1], f32, tag="rden", bufs=1)
        nc.vector.reciprocal(rden[:], den[:])
        g = vp.tile([BL, 8], f32, tag="g", bufs=1)
        nc.vector.tensor_scalar(g[:], lg[:], m2[:], None,
                                op0=mybir.AluOpType.is_ge)
        zg = vp.tile([BL, 8], f32, tag="zg", bufs=1)
        nc.vector.tensor_tensor(zg[:], z[:], g[:], mybir.AluOpType.mult)
        coef = vp.tile([BL, 8], bf, tag="coef", bufs=1)
        nc.vector.tensor_scalar_mul(coef[:], zg[:], rden[:])
        # coef [16 tok, 8 e] bf16 -> payload row 98 (16 bytes per token)
        nc.vector.tensor_copy(
            fz[0:BL, 98:99, :].rearrange("p r t -> p (r t)"),
            coef[:].bitcast(f8))
        nc.scalar.dma_start(ag_d[3][:], fz[:, 84:102, :])
        nc.gpsimd.collective_compute(
            "AllGather", mybir.AluOpType.bypass, replica_groups=RG,
            ins=[ag_d[3][:]], outs=[cc_d[3][:]])

        # ---------------- stage featT + coef columns
        SQ = (nc.sync, nc.scalar)
        for r in range(3, NCORE):
            SQ[r % 2].dma_start(
                featT[:, r, 14:28],
                cc_d[1][r].rearrange("c (s l) t -> c s l t", l=2))
        for r in range(NCORE):
            SQ[r % 2].dma_start(
                featT[:, r, 28:42],
                cc_d[2][r].rearrange("c (s l) t -> c s l t", l=2))
        for r in range(NCORE):
            SQ[r % 2].dma_start(
                featT[:, r, 42:49],
                cc_d[3][r, :, 0:14].rearrange("c (s l) t -> c s l t", l=2))
        coef_all = vp.tile([128, 8], bf, tag="coefall", bufs=1)
        CQ = (nc.sync, nc.scalar, nc.sync, nc.scalar)
        for r in range(NCORE):
            CQ[r % 4].dma_start(
                coef_all[BL * r:BL * r + BL, :].bitcast(f8),
                cc_d[3][r, 0:BL, 14:15, :].rearrange("p r t -> p (r t)"))
        coef_f = vp.tile([128, 8], f32, tag="coeff", bufs=1)
        nc.vector.tensor_copy(coef_f[:], coef_all[:])
        cm = vp.tile([128, 8], f32, tag="cm", bufs=1)
        nc.vector.tensor_tensor(cm[:], coef_f[:], oht[:], mybir.AluOpType.mult)
        coefe = vp.tile([128, 1], f32, tag="coefe", bufs=1)
        nc.vector.tensor_reduce(coefe[:], cm[:], axis=mybir.AxisListType.X,
                                op=ADD)

        # ---------------- expert MLP: 3term DR over s-pairs
        nc.tensor.matmul(accum[:].rearrange("p m h -> p (m h)"),
                         zf8[:, 0:128], zf8[:], start=True, stop=False,
                         skip_group_check=True)

        def exp_stage(m, q0, q1, first, last):
            for q in range(q0, q1):
                s = 2 * q
                for r in range(NCORE):
                    dst = accum[:, m, BL * r:BL * r + BL]
                    nc.tensor.matmul(dst, ew1h[:, s:s + 2, m, :],
                                     featT[:, r, s:s + 2, 0], start=False,
                                     stop=False, perf_mode=DR,
                                     skip_group_check=True)
                    nc.tensor.matmul(dst, ew1h[:, s:s + 2, m, :],
                                     featT[:, r, s:s + 2, 1], start=False,
                                     stop=False, perf_mode=DR,
                                     skip_group_check=True)
                    nc.tensor.matmul(dst, ew1l[:, s:s + 2, m, :],
                                     featT[:, r, s:s + 2, 0], start=False,
                                     stop=False, perf_mode=DR,
                                     skip_group_check=True)

        for m in range(4):
            exp_stage(m, 0, 14, True, False)      # s 0..27 (chunks 1+2)
        for m in range(4):
            exp_stage(m, 14, 21, False, False)    # s 28..41 (chunk 3)
        for m in range(4):
            exp_stage(m, 21, 24, False, False)    # s 42..47
            for r in range(NCORE):
                dst = accum[:, m, BL * r:BL * r + BL]
                nc.tensor.matmul(dst, ew1h[:, 48, m, :], featT[:, r, 48, 0],
                                 start=False, stop=False, skip_group_check=True)
                nc.tensor.matmul(dst, ew1h[:, 48, m, :], featT[:, r, 48, 1],
                                 start=False, stop=False, skip_group_check=True)
                nc.tensor.matmul(dst, ew1l[:, 48, m, :], featT[:, r, 48, 0],
                                 start=False, stop=(m == 3),
                                 skip_group_check=True)
        hid = pers.tile([128, 4, 128], bf)
        for m in range(4):
            nc.scalar.activation(out=hid[:, m, :], in_=accum[:, m, :],
                                 func=RELU, bias=eb1t[:, m:m + 1],
                                 scale=esit[:, m:m + 1])
        sqpre = vp.tile([1, 1], f32, tag="sqpre", bufs=1)
        nc.scalar.activation(out=sqpre[:], in_=esit[0:1, 0:1],
                             func=mybir.ActivationFunctionType.Sqrt, scale=1.0)
        for m in range(4):
            nc.tensor.matmul(ps2[:], hid[:, m, :], ew2t[:, m, :],
                             start=(m == 0), stop=False)
        nc.tensor.matmul(ps2[:], ones[:], eb2t[:], start=False, stop=True)
        wout = vp.tile([128, 256], f32, tag="wout", bufs=1)
        nc.vector.tensor_scalar_mul(wout[:], ps2[:], coefe[:])
        nc.sync.dma_start(rs_in[:], wout[:])

        # ---------------- ReduceScatter + L2 normalize token shard
        nc.gpsimd.collective_compute(
            "ReduceScatter", mybir.AluOpType.add, replica_groups=RG,
            ins=[rs_in[:]], outs=[rs_out[:]])
        if debug:
            nc.sync.dma_start(dbg_pool1[:], pool1[:])
            nc.sync.dma_start(dbg_pool2[:], pool2[:])
            nc.sync.dma_start(dbg_feat[:], feat_sb[:])
            nc.sync.dma_start(dbg_fz[:], fz[:])
            nc.sync.dma_start(dbg_lg[:], lg[:])
            nc.sync.dma_start(dbg_coefe[:], coefe[:])
            nc.sync.dma_start(dbg_hid[:], hid[:])
            nc.sync.dma_start(dbg_featT[:], featT[:])
        nsb = vp.tile([BL, 256], f32, tag="nsb", bufs=1)
        nc.sync.dma_start(nsb[:], rs_out[:])
        sq = vp.tile([BL, 256], f32, tag="sq", bufs=1)
        ss = vp.tile([BL, 1], f32, tag="ss", bufs=1)
        nc.scalar.activation(out=sq[:], in_=nsb[:],
                             func=mybir.ActivationFunctionType.Square,
                             accum_out=ss[:])
        nrm = vp.tile([BL, 1], f32, tag="nrm", bufs=1)
        nc.scalar.activation(out=nrm[:], in_=ss[:],
                             func=mybir.ActivationFunctionType.Sqrt, scale=1.0)
        nc.vector.tensor_scalar_max(nrm[:], nrm[:], 1e-12)
        rn = vp.tile([BL, 1], f32, tag="rn", bufs=1)
        nc.vector.reciprocal(rn[:], nrm[:])
        yt = vp.tile([BL, 256], f32, tag="yt", bufs=1)
        nc.vector.tensor_scalar_mul(yt[:], nsb[:], rn[:])
        nc.sync.dma_start(y_d[:], yt[:])
    nc.compile()
    return nc


# ---------------------------------------------------------------------------
# host orchestration
# ---------------------------------------------------------------------------

_CACHE = {}
LAST = {}


def _prep_inputs(inputs):
    w1, bb1 = _fold_bn(inputs["conv1_w"], inputs["conv1_b"], inputs["bn1_g"],
                       inputs["bn1_b"], inputs["bn1_m"], inputs["bn1_v"])
    w2, bb2 = _fold_bn(inputs["conv2_w"], inputs["conv2_b"], inputs["bn2_g"],
                       inputs["bn2_b"], inputs["bn2_m"], inputs["bn2_v"])
    w3, bb3 = _fold_bn(inputs["conv3_w"], inputs["conv3_b"], inputs["bn3_g"],
                       inputs["bn3_b"], inputs["bn3_m"], inputs["bn3_v"])
    w1hh, w1lh, s1inv = _l1_weights(w1)
    w2hh, w2lh, s2inv = _l2_weights(w2)
    w3hh, w3lh, s3inv = _l3_weights(w3)
    rw = inputs["router_w"]
    shared = {
        "w1hh": w1hh, "w1lh": w1lh, "w2hh": w2hh, "w2lh": w2lh,
        "w3hh": w3hh, "w3lh": w3lh,
        "b1": np.tile(bb1.astype(np.float32), 3)[:, None],
        "b2": np.tile(bb2.astype(np.float32), 2)[:, None],
        "b3": bb3.astype(np.float32)[:, None],
        "s1inv": s1inv, "s2inv": s2inv, "s3inv": s3inv,
        "rwt": np.ascontiguousarray(
            (0.25 * rw.T).reshape(128, 49, 8).astype(BF16)),
        "rb": inputs["router_b"].astype(np.float32)[None, :],
    }
    in_maps = []
    for e in range(NCORE):
        m = dict(shared)
        m["xblk"] = _l1_blocks(np.asarray(inputs["x"][e * BL:(e + 1) * BL],
                                          dtype=np.float32))
        ew1h, ew1l, esi, ew2t, eb1t, eb2r = _expert_weights(
            inputs["ew1"][e], inputs["eb1"][e], inputs["ew2"][e],
            inputs["eb2"][e])
        oh = np.zeros((128, 8), np.float32)
        oh[:, e] = 1.0
        m.update({"ew1h": ew1h, "ew1l": ew1l, "esi": esi, "ew2": ew2t,
                  "eb1": eb1t, "eb2r": eb2r, "oh": oh})
        in_maps.append(m)
    return in_maps


def kernel(**inputs):
    inputs = {k: np.asarray(v) for k, v in inputs.items()}
    in_maps = _prep_inputs(inputs)
    if "fused_v2" not in _CACHE:
        _CACHE["fused_v2"] = build_fused_v2()
    res = run_bass_kernel_spmd(_CACHE["fused_v2"], in_maps,
                               core_ids=list(range(NCORE)))
    LAST["a"] = res
    return np.concatenate([res.results[c]["y"] for c in range(NCORE)], axis=0)
